# revision 4
# baseline (speedup 1.0000x reference)
"""Complex multi-head attention (B=4, S=2048, D=512, H=8) on 8 TRN2 NeuronCores.

Sharding: core c handles batch b = c//2 and head group hg = c%2 (4 heads each).
Weights are head-sliced host-side; each core computes its 4 heads' attention and
a partial output projection; the host sums the two partials per batch.

v3 schedule: the kernel is tensor-engine bound overall (every matmul is
[K<=128 x 128 x 512f] at ~213ns) but softmax-exp saturates the Scalar (ACT)
engine at ~1.15us per [128,1024] chunk, which makes each attention iteration
ACT-paced.  So:
  - the softmax denominator is tree-summed on the Vector engine (15 bf16 adds)
    down to ONE [128,1024] tile, needing only 2 ones-matmuls per iteration
    instead of 16 (PE work per iter drops 17.0 -> 14.1us, below the ACT pace);
  - the V projection uses the 3-multiplication Gauss trick for the complex
    product (M1=Xr@Wr, M2=Xi@Wi, M3=(Xr+Xi)@(Wr+Wi); Re=M1-M2, Im=M3-M1-M2,
    combined on the Vector engine in token-major layout);
  - the Q/K projections for heads 2,3 and the whole output projection are
    emitted as FILLER matmuls inside the attention iterations' ACT-wait slack,
    so the tensor engine never idles while exp streams.
All matmuls bf16 with f32 PSUM accumulation; output is stored bf16 (partials
are summed in f32 on host). exp is computed without max subtraction
(|scores| <= ~18 on this distribution).
"""

import os

import numpy as np

import concourse.mybir as mybir
import concourse.tile as tile
from concourse import bacc
from concourse.bass import ds, ts
from concourse.bass_utils import run_bass_kernel_spmd

F32 = mybir.dt.float32
BF16 = mybir.dt.bfloat16

B, S, D = 4, 2048, 512
H, Dh = 8, 64
HPC = 4          # heads per core
SCALE = 1.0 / 8.0  # 1/sqrt(Dh)

_NC = None


def _build():
    nc = bacc.Bacc("TRN2", target_bir_lowering=False, debug=False, num_devices=8)

    # xt chunks 0-7: [x.T.re ; x.T.im] rows (contraction 1024); 8-11: (re+im)
    # rows for the V-projection's Gauss M3 term.
    xt_d = nc.declare_dram_parameter("xt", [128, 12, S], BF16, isOutput=False)
    wq_d = nc.declare_dram_parameter("wq", [128, 8, HPC, 128], BF16, isOutput=False)
    wk_d = nc.declare_dram_parameter("wk", [128, 8, HPC, 128], BF16, isOutput=False)
    # Gauss V weights: [f_local, cc, m in (re, im, re+im), h*64+j]
    wvg_d = nc.declare_dram_parameter("wvg", [128, 4, 3, 256], BF16, isOutput=False)
    r_d = nc.declare_dram_parameter("r", [128, HPC, 1024], BF16, isOutput=False)
    ones_d = nc.declare_dram_parameter("ones", [128, 128], BF16, isOutput=False)
    out_d = nc.declare_dram_parameter("out", [S, 1024], BF16, isOutput=True)

    Exp = mybir.ActivationFunctionType.Exp

    with tile.TileContext(nc) as tc:
        with tc.tile_pool(name="sb", bufs=1) as sb:
            ones = sb.tile([128, 128], BF16)
            xt_s = sb.tile([128, 12, S], BF16)
            wq_s = sb.tile([128, 8, HPC, 128], BF16)
            wk_s = sb.tile([128, 8, HPC, 128], BF16)
            wvg_s = sb.tile([128, 4, 3, 256], BF16)
            r_s = sb.tile([128, HPC, 1024], BF16)
            # per-head tiles so interleaved writers (filler QK waves, late
            # normalization) never alias the tiles the attention loop reads
            qts = [sb.tile([128, S], BF16, name=f"qt{hh}") for hh in range(HPC)]
            kts = [sb.tile([128, S], BF16, name=f"kt{hh}") for hh in range(HPC)]
            v = sb.tile([128, 16, 512], BF16)  # [k%128, k//128, h*128+(re|im)*64+j]
            ots = [sb.tile([128, S], BF16, name=f"ot{hh}") for hh in range(HPC)]

            # ---- input DMAs: critical-path first.  The first Q-wave matmul
            # needs only wq[c=0,h01] + xt[0, tg0], so those go out as tiny
            # transfers before the bulk.  Heads 2,3 weights are filler-phase
            # (iter 0-4) and ones/r are late: load them last.  gpsimd (slow
            # software queue) carries wvg (V phase, ~t+40us) and r (iter 6+).
            nc.scalar.dma_start(out=wq_s[:, 0:2, 0:2, :], in_=wq_d[:, 0:2, 0:2, :])
            nc.sync.dma_start(out=xt_s[:, 0, 0:512], in_=xt_d[:, 0, 0:512])
            nc.scalar.dma_start(out=wq_s[:, 2:5, 0:2, :], in_=wq_d[:, 2:5, 0:2, :])
            nc.sync.dma_start(out=xt_s[:, 0, 512:2048], in_=xt_d[:, 0, 512:2048])
            nc.gpsimd.dma_start(out=wvg_s[:, :, :, :], in_=wvg_d[:, :, :, :])
            nc.scalar.dma_start(out=wq_s[:, 5:8, 0:2, :], in_=wq_d[:, 5:8, 0:2, :])
            nc.sync.dma_start(out=xt_s[:, 1, :], in_=xt_d[:, 1, :])
            nc.scalar.dma_start(out=wk_s[:, 0:3, 0:2, :], in_=wk_d[:, 0:3, 0:2, :])
            nc.sync.dma_start(out=xt_s[:, 2, :], in_=xt_d[:, 2, :])
            nc.scalar.dma_start(out=wk_s[:, 3:8, 0:2, :], in_=wk_d[:, 3:8, 0:2, :])
            nc.sync.dma_start(out=xt_s[:, 3, :], in_=xt_d[:, 3, :])
            nc.scalar.dma_start(out=xt_s[:, 4:6, :], in_=xt_d[:, 4:6, :])
            nc.sync.dma_start(out=xt_s[:, 6:8, :], in_=xt_d[:, 6:8, :])
            nc.scalar.dma_start(out=xt_s[:, 10:12, :], in_=xt_d[:, 10:12, :])
            nc.sync.dma_start(out=xt_s[:, 8:10, :], in_=xt_d[:, 8:10, :])
            nc.scalar.dma_start(out=wq_s[:, :, 2:4, :], in_=wq_d[:, :, 2:4, :])
            nc.sync.dma_start(out=wk_s[:, :, 2:4, :], in_=wk_d[:, :, 2:4, :])
            nc.sync.dma_start(out=ones[:, :], in_=ones_d[:, :])
            nc.gpsimd.dma_start(out=r_s[:, :, :], in_=r_d[:, :, :])

            def qk_wave(dst, w_s, heads, pool, group_major=False):
                # c-major: 8 concurrent PSUM chains so every arriving xt chunk
                # feeds 8 matmuls during the initial DMA.  group-major (data
                # resident): drain each group right after its chain so the
                # last drain lands with the last matmul, not 8 casts later.
                groups = [(hh, tg) for hh in heads for tg in range(4)]
                if group_major:
                    for gi, (hh, tg) in enumerate(groups):
                        gt = pool.tile([128, 512], F32, name=f"acc{gi}", tag=f"acc{gi}")
                        for c in range(8):
                            nc.tensor.matmul(
                                gt[:, :],
                                lhsT=w_s[:, c, hh, :],
                                rhs=xt_s[:, c, ts(tg, 512)],
                                start=(c == 0),
                                stop=(c == 7),
                            )
                        nc.vector.tensor_copy(out=dst[hh][:, ts(tg, 512)], in_=gt[:, :])
                    return
                tiles = [
                    pool.tile([128, 512], F32, name=f"acc{gi}", tag=f"acc{gi}")
                    for gi in range(len(groups))
                ]
                for c in range(8):
                    for gi, (hh, tg) in enumerate(groups):
                        nc.tensor.matmul(
                            tiles[gi][:, :],
                            lhsT=w_s[:, c, hh, :],
                            rhs=xt_s[:, c, ts(tg, 512)],
                            start=(c == 0),
                            stop=(c == 7),
                        )
                for gi, (hh, tg) in enumerate(groups):
                    nc.vector.tensor_copy(out=dst[hh][:, ts(tg, 512)], in_=tiles[gi][:, :])

            # ---- Q and K for heads 0,1 (heads 2,3 are filler later) ----
            with tc.tile_pool(name="w8", bufs=1, space="PSUM") as w8:
                qk_wave(qts, wq_s, (0, 1), w8)
                qk_wave(kts, wk_s, (0, 1), w8, group_major=True)

            # ---- V projection, Gauss 3-mult ----
            with (
                tc.tile_pool(name="vg", bufs=2, space="PSUM") as vgp,
                tc.tile_pool(name="vt", bufs=2) as vtp,
            ):
                for tb in range(16):
                    ms = [vgp.tile([128, 512], F32, name=f"m{mi}", tag=f"m{mi}")
                          for mi in range(3)]
                    for mi in range(3):
                        for cc in range(4):
                            c = cc + (0, 4, 8)[mi]
                            nc.tensor.matmul(
                                ms[mi][:, 0:256],
                                lhsT=xt_s[:, c, ts(tb, 128)],
                                rhs=wvg_s[:, cc, mi, :],
                                start=(cc == 0),
                                stop=(cc == 3),
                            )
                    # DVE has 2 SBUF read ports: keep <=1 PSUM operand
                    # per op.  a1 = M1 (SBUF), tmp = M1+M2 (SBUF), then
                    # Re = a1 - M2 and Im = M3 - tmp per head.
                    a1 = vtp.tile([128, 256], F32, name="a1")
                    tmp = vtp.tile([128, 256], F32, name="tmp")
                    nc.scalar.copy(out=a1[:, :], in_=ms[0][:, 0:256])
                    nc.vector.tensor_add(tmp[:, :], a1[:, :], ms[1][:, 0:256])
                    for hh in range(HPC):
                        nc.vector.tensor_sub(
                            v[:, tb, ds(hh * 128, 64)],
                            a1[:, ds(hh * 64, 64)],
                            ms[1][:, ds(hh * 64, 64)],
                        )
                        nc.vector.tensor_sub(
                            v[:, tb, ds(hh * 128 + 64, 64)],
                            ms[2][:, ds(hh * 64, 64)],
                            tmp[:, ds(hh * 64, 64)],
                        )

            # ---- attention + interleaved filler (QK heads 2,3 + out proj) ----
            with (
                tc.tile_pool(name="st", bufs=2, space="PSUM") as stp,
                tc.tile_pool(name="ov", bufs=1, space="PSUM") as ovp,
                tc.tile_pool(name="fq", bufs=2, space="PSUM") as fqp,
                tc.tile_pool(name="pt", bufs=4) as ptp,
                tc.tile_pool(name="pr", bufs=3) as prp,
                tc.tile_pool(name="qd", bufs=3) as qdp,
                tc.tile_pool(name="misc", bufs=2) as miscp,
                tc.tile_pool(name="or", bufs=2) as orp,
                tc.tile_pool(name="ysb", bufs=3) as ysb,
            ):
                # filler units: each closure emits ONE PE matmul (plus its
                # trailing drain when it ends an accumulation chain).
                filler = []

                def qk_group_units(w_s, dst, hh, tg):
                    box = {}

                    def mm(c):
                        if c == 0:
                            box["t"] = fqp.tile([128, 512], F32, name="qkf")
                        nc.tensor.matmul(
                            box["t"][:, :],
                            lhsT=w_s[:, c, hh, :],
                            rhs=xt_s[:, c, ts(tg, 512)],
                            start=(c == 0),
                            stop=(c == 7),
                        )
                        if c == 7:
                            nc.vector.tensor_copy(
                                out=dst[hh][:, ts(tg, 512)], in_=box["t"][:, :]
                            )

                    return [lambda c=c: mm(c) for c in range(8)]

                def o_tb_units(tb):
                    box = {}

                    def mm(g, hc):
                        if g == 0 and hc == 0:
                            box[0] = fqp.tile([128, 512], F32, name="qkf")
                            box[1] = fqp.tile([128, 512], F32, name="qkf")
                        nc.tensor.matmul(
                            box[g][:, :],
                            lhsT=ots[hc][:, ts(tb, 128)],
                            rhs=r_s[:, hc, ts(g, 512)],
                            start=(hc == 0),
                            stop=(hc == 3),
                        )
                        if g == 1 and hc == 3:
                            y_s = ysb.tile([128, 1024], BF16)
                            nc.vector.tensor_copy(out=y_s[:, 0:512], in_=box[0][:, :])
                            nc.vector.tensor_copy(out=y_s[:, 512:1024], in_=box[1][:, :])
                            # keep the scalar queue exp-only during attention:
                            # out DMAs go on sync/gpsimd (both idle here)
                            q_ = nc.sync if tb % 2 == 0 else nc.gpsimd
                            q_.dma_start(out=out_d[ts(tb, 128), :], in_=y_s[:, :])

                    return [lambda g=g, hc=hc: mm(g, hc) for g in range(2) for hc in range(HPC)]

                # deadline order: h2 needed at iter 4, h3 at iter 5
                for hh in (2, 3):
                    for w_s_, dst in ((wq_s, qts), (wk_s, kts)):
                        for tg in range(4):
                            filler.extend(qk_group_units(w_s_, dst, hh, tg))

                iters = [(0, 0), (0, 1), (1, 0), (1, 1), (2, 0), (3, 0), (2, 1), (3, 1)]

                def fill_n(it, kc):
                    # QK23 (128 units) over iters 0-4 at ~1.75/chunk, done by
                    # iter-4 chunk 8 (kt[h3] ready just before iter 5); the out
                    # projection tb0-7 (64 units) over iter-6 kc>=4 + iter 7
                    # (kc<4 of iter 6 must stay empty: those units read ot
                    # written by norms that are only emitted at kc==3).
                    # Cap at 2/chunk so PE never outpaces the exp stream by
                    # more than ~80ns/chunk (3 fillers = 7 MMs = 1.5us versus
                    # the 1.22us ACT chunk pace, which stalls exp).
                    if it < 4:
                        return 2 if (kc % 4) != 3 else 1
                    if it == 4:
                        return 2 if kc < 8 else 0
                    if it == 5:
                        return 0
                    if it == 6:
                        return 0 if kc < 4 else 2
                    return 3 if kc < 8 else 2
                pending = [None]
                fi = 0
                for it, (h, qh) in enumerate(iters):
                    if it == 6:
                        for tb in range(8):
                            filler.extend(o_tb_units(tb))
                    o_halves = (
                        ovp.tile([128, 512], F32, name="o0", tag="o0"),
                        ovp.tile([128, 512], F32, name="o1", tag="o1"),
                    )
                    pts, pairs, quads, octs = [], [], [], []
                    oraw = orp.tile([128, 1024], BF16, name="oraw")
                    def emit_scores(kc):
                        st_t = stp.tile([128, 1024], F32, name="st_t")
                        for g in range(2):
                            nc.tensor.matmul(
                                st_t[:, ts(g, 512)],
                                lhsT=kts[h][:, ts(kc, 128)],
                                rhs=qts[h][:, ds(qh * 1024 + g * 512, 512)],
                                start=True,
                                stop=True,
                            )
                        pt_t = ptp.tile([128, 1024], BF16)
                        nc.scalar.activation(
                            out=pt_t[:, :], in_=st_t[:, :], func=Exp, scale=SCALE
                        )
                        pts.append(pt_t)

                    emit_scores(0)
                    for kc in range(16):
                        # scores for the NEXT chunk go ahead of this chunk's
                        # exp-dependent AV matmuls (keeps ACT streaming and
                        # gives the PE queue work while exp(kc) runs).
                        if kc + 1 < 16:
                            emit_scores(kc + 1)
                        for _ in range(fill_n(it, kc)):
                            if fi < len(filler):
                                filler[fi]()
                                fi += 1
                        if kc == 3 and pending[0] is not None:
                            pending[0]()
                            pending[0] = None
                        pt_t = pts[kc]
                        for g in range(2):
                            nc.tensor.matmul(
                                o_halves[g][:, :],
                                lhsT=v[:, kc, ds(h * 128, 128)],
                                rhs=pt_t[:, ts(g, 512)],
                                start=(kc == 0),
                                stop=(kc == 15),
                            )
                        if kc == 15:
                            # drain o ahead of the last tree adds: the next
                            # iteration's first AV then only waits on these
                            # two casts, not on the d/recip/mul chain.
                            nc.vector.tensor_copy(
                                out=oraw[:, 0:512], in_=o_halves[0][:, :]
                            )
                            nc.vector.tensor_copy(
                                out=oraw[:, 512:1024], in_=o_halves[1][:, :]
                            )
                        # denominator: pair-sum then running-sum on the
                        # Vector engine -- the final sum is ready one add
                        # after the last exp (short cross-engine tail).
                        if kc % 2 == 1:
                            pr = prp.tile([128, 1024], BF16)
                            nc.vector.tensor_add(pr[:, :], pts[kc - 1][:, :], pts[kc][:, :])
                            if not pairs:
                                pairs.append(pr)
                            else:
                                rn = qdp.tile([128, 1024], BF16, name="run")
                                nc.vector.tensor_add(
                                    rn[:, :], pairs[-1][:, :], pr[:, :]
                                )
                                pairs.append(rn)
                    fin = pairs[-1]

                    def norm(h=h, qh=qh, fin=fin, oraw=oraw):
                        # deferred: emitted a few chunks into the NEXT
                        # iteration so the d->recip->mul chain never stalls
                        # the tensor engine.
                        d_t = stp.tile([128, 1024], F32, name="st_t")
                        for g in range(2):
                            nc.tensor.matmul(
                                d_t[:, ts(g, 512)],
                                lhsT=ones[:, :],
                                rhs=fin[:, ts(g, 512)],
                                start=True,
                                stop=True,
                            )
                        rec = miscp.tile([128, 1024], F32)
                        nc.vector.reciprocal_approx_fast(out=rec[:, :], in_=d_t[:, :])
                        for g in range(2):
                            nc.vector.tensor_mul(
                                ots[h][:, ds(qh * 1024 + g * 512, 512)],
                                oraw[:, ts(g, 512)],
                                rec[:, ts(g, 512)],
                            )

                    pending[0] = norm
                pending[0]()
                pending[0] = None
                # drain leftover filler (none expected), then tail: out proj
                # for token blocks 8-15.
                while fi < len(filler):
                    filler[fi]()
                    fi += 1
                for tb in range(8, 16):
                    for u in o_tb_units(tb):
                        u()

    nc.compile()
    return nc


def _wcat_head(w_h):
    """[64, 512] complex head-slice of a projection weight -> [1024, 128] real
    stationary block: out column j<64 produces re(head feature j), j>=64 im."""
    wr = np.ascontiguousarray(w_h.real).astype(np.float32)
    wi = np.ascontiguousarray(w_h.imag).astype(np.float32)
    top = np.concatenate([wr.T, wi.T], axis=1)     # x_re rows
    bot = np.concatenate([-wi.T, wr.T], axis=1)    # x_im rows
    return np.concatenate([top, bot], axis=0)      # [1024, 128]


def _core_inputs(x, wq, wk, wv, wo, core):
    import ml_dtypes

    b, hg = divmod(core, 2)
    heads = [hg * HPC + h for h in range(HPC)]

    xr = x[b].T.real.astype(np.float32)   # [512, 2048]
    xi = x[b].T.imag.astype(np.float32)
    xt = np.concatenate([xr, xi, xr + xi], axis=0)  # [1536, 2048]
    xt = np.ascontiguousarray(xt.reshape(12, 128, S).transpose(1, 0, 2))

    def _wqk(w):
        blocks = np.stack(
            [_wcat_head(w[gh * Dh : (gh + 1) * Dh]) for gh in heads]
        )  # [4, 1024, 128]
        return np.ascontiguousarray(
            blocks.reshape(HPC, 8, 128, 128).transpose(2, 1, 0, 3)
        )  # [128, 8, 4, 128]

    # Gauss V weights: [512, 256] per m, chunked on contraction
    wvr = np.concatenate(
        [wv[gh * Dh : (gh + 1) * Dh].real.T.astype(np.float32) for gh in heads],
        axis=1,
    )  # [512, 256]
    wvi = np.concatenate(
        [wv[gh * Dh : (gh + 1) * Dh].imag.T.astype(np.float32) for gh in heads],
        axis=1,
    )
    wvg = np.stack([wvr, wvi, wvr + wvi], axis=1)  # [512, 3, 256]
    wvg = np.ascontiguousarray(
        wvg.reshape(4, 128, 3, 256).transpose(1, 0, 2, 3)
    )  # [128, 4, 3, 256]

    r_blocks = []
    for gh in heads:
        wo_h = wo[:, gh * Dh : (gh + 1) * Dh]  # [512, 64] complex
        wor = np.ascontiguousarray(wo_h.real).astype(np.float32)
        woi = np.ascontiguousarray(wo_h.imag).astype(np.float32)
        top = np.concatenate([wor.T, woi.T], axis=1)    # O_re rows -> [64, 1024]
        bot = np.concatenate([-woi.T, wor.T], axis=1)   # O_im rows
        r_blocks.append(np.concatenate([top, bot], axis=0))  # [128, 1024]
    r_cat = np.concatenate(r_blocks, axis=0)  # [512, 1024]
    r_cat = np.ascontiguousarray(r_cat.reshape(HPC, 128, 1024).transpose(1, 0, 2))

    out = {
        "xt": xt,
        "wq": _wqk(wq),
        "wk": _wqk(wk),
        "wvg": wvg,
        "r": r_cat,
        "ones": np.ones((128, 128), dtype=np.float32),
    }
    return {k: v.astype(ml_dtypes.bfloat16) for k, v in out.items()}


def kernel(x, wq, wk, wv, wo):
    global _NC
    x = np.asarray(x)
    wq = np.asarray(wq)
    wk = np.asarray(wk)
    wv = np.asarray(wv)
    wo = np.asarray(wo)

    if _NC is None:
        _NC = _build()

    in_maps = [_core_inputs(x, wq, wk, wv, wo, c) for c in range(8)]

    trace = os.environ.get("KERNEL_PROFILE", "0") == "1"
    kwargs = {}
    if trace:
        _install_profile_shim()
        kwargs = {"trace": True}
    res = run_bass_kernel_spmd(_NC, in_maps, core_ids=list(range(8)), **kwargs)
    if trace:
        print(f"HW exec time: {res.exec_time_ns} ns")

    out = np.zeros((B, S, D), dtype=np.complex64)
    for c in range(8):
        b = c // 2
        y = np.asarray(res.results[c]["out"]).astype(np.float32)
        out[b] += y[:, :512] + 1j * y[:, 512:]
    return out


def _install_profile_shim():
    """Register the NTFF profile hook for axon (missing antenv.axon_hooks)."""
    import contextlib
    import ctypes
    import sys
    import types

    try:
        import antenv.axon_hooks  # noqa: F401

        return
    except ImportError:
        pass

    so_path = "/opt/axon/libaxon_pjrt.so"
    lib = ctypes.CDLL(so_path)
    if not hasattr(lib, "axon_start_nrt_profile"):
        return
    lib.axon_start_nrt_profile.argtypes = [
        ctypes.POINTER(ctypes.c_int64),
        ctypes.c_size_t,
    ]
    lib.axon_start_nrt_profile.restype = ctypes.c_int64
    lib.axon_stop_nrt_profile.argtypes = [ctypes.c_char_p]
    lib.axon_stop_nrt_profile.restype = ctypes.c_int64

    @contextlib.contextmanager
    def _hook(output_dir, device_ids):
        import jax

        jax.devices()
        if device_ids:
            ids = (ctypes.c_int64 * len(device_ids))(*device_ids)
            rc = lib.axon_start_nrt_profile(ids, len(device_ids))
        else:
            rc = lib.axon_start_nrt_profile(None, 0)
        if rc != 0:
            raise RuntimeError(f"axon_start_nrt_profile rc={rc}")
        try:
            yield
        finally:
            n = lib.axon_stop_nrt_profile(str(output_dir).encode())
            print(f"profile: {n} file(s) -> {output_dir}", file=sys.stderr)

    mod = types.ModuleType("antenv.axon_hooks")
    _h = [_hook]

    mod.set_axon_ntff_profile_hook = lambda h: _h.__setitem__(0, h)
    mod.get_axon_ntff_profile_hook = lambda: _h[0]
    sys.modules["antenv.axon_hooks"] = mod
    import antenv

    antenv.axon_hooks = mod

    import concourse.bass_utils as bu

    bu.upload_artifacts = lambda tmpdir: str(tmpdir)

